# revision 25
# baseline (speedup 1.0000x reference)
"""Trainium2 Bass kernel for Bahdanau-style attention (nn_Attention).

Reference computation (B=128, S=1024, D=512):
    proj = tanh(concat(dec, enc) @ W1.T + b1)        # [B, S, D]
    scores = proj @ W2.T (+ b2, cancels in softmax)  # [B, S]
    alpha = softmax(scores, axis=1)
    context = einsum('bs,bsd->bd', alpha, enc)       # [B, D]

Strategy: pure data-parallel over batch (16 rows per NeuronCore, 8 cores).
Per-core dataflow:
  - h-channels are permuted by |w2| (host side): the top-128 rows land in
    chunk hc0 and are computed in fp16; chunks hc1-3 run in fp8-e4m3 with
    DoubleRow perf mode (contracts 256/matmul).  Since proj error reaches
    the output only through scores = sum_h w2_h tanh(proj_h), and the top
    128 |w2| rows carry ~71% of sum(w2^2), this cuts the fp8-induced
    error ~2x (verified rel err ~1.45e-2 vs the 2e-2 budget on the
    harness input set; all-fp8 measured 2.6e-2).
  - encT is DMAed once in fp16; the fp8 copy for the DoubleRow chunks is
    produced on-chip by an idle-DVE tensor_copy (saves 8.4MB of HBM
    traffic).
  - (proj_dec[b] + b1) becomes a per-partition bias fused into the
    ScalarE tanh that evacuates PSUM.
  - scores = W2 . hiddenT via M=1 PE matmuls issued round-robin across
    the four 32-wide PE column groups (tile_position) so the 4 batch
    rows of a softmax group stream concurrently.
  - softmax batched over groups of 4 batch rows (partitions 0/32/64/96):
    Exp reads the scores PSUM directly on ScalarE with a fused accum_out
    denominator; max-subtraction skipped (scores are O(+-3) here).
  - unnormalized alpha (fp16) transposed via PE transpose; context =
    alphaT^T @ enc_natural via col-tiled M=1 matmuls, also round-robin,
    scaled by 1/den at evacuation.
Host side: shard batch, permute h by |w2|, pre-transpose enc to encT
(fp16) + encN (fp16).
"""

import numpy as np
import ml_dtypes

B, S, D = 128, 1024, 512
N_CORES = 8
B_LOC = B // N_CORES          # 16
GB = 4                        # batch rows per softmax group
NG = B_LOC // GB              # 4 groups
DC = D // 128                 # 4 chunks of 128 along d (and h)
F16C = 1                      # h-chunks computed in fp16 (rest fp8 DR)
SBLK = 512                    # s block for proj/score tiles
NSB = S // SBLK               # 2
NSC = S // 128                # 8 s-chunks of 128

_NPF16 = np.float16
_NPF8 = ml_dtypes.float8_e4m3     # TRN float8e4 (max 240, matches ml_dtypes)
_CACHE: dict = {}


def _build():
    from contextlib import ExitStack
    import concourse.bass as bass  # noqa: F401
    import concourse.tile as tile
    from concourse import bacc, mybir

    f32, f16 = mybir.dt.float32, mybir.dt.float16
    fp8 = mybir.dt.float8e4
    AX = mybir.AxisListType
    OP = mybir.AluOpType
    AF = mybir.ActivationFunctionType
    DR = mybir.MatmulPerfMode.DoubleRow

    nc = bacc.Bacc("TRN2", target_bir_lowering=False, debug=False,
                   num_devices=N_CORES)

    encT = nc.dram_tensor("encT", [B_LOC, NSB, 128, DC, SBLK], f16, kind="ExternalInput").ap()
    encN = nc.dram_tensor("encN", [B_LOC, 128, NSC, D], f16, kind="ExternalInput").ap()
    w1eT16 = nc.dram_tensor("w1eT16", [F16C, 128, DC, 128], f16, kind="ExternalInput").ap()
    w1eT8 = nc.dram_tensor("w1eT8", [DC - F16C, 128, DC, 128], fp8, kind="ExternalInput").ap()
    w1dT16 = nc.dram_tensor("w1dT16", [F16C, 128, DC, 128], f16, kind="ExternalInput").ap()
    w1dT8 = nc.dram_tensor("w1dT8", [DC - F16C, 128, DC, 128], fp8, kind="ExternalInput").ap()
    decT16 = nc.dram_tensor("decT16", [DC, 128, B_LOC], f16, kind="ExternalInput").ap()
    decT8 = nc.dram_tensor("decT8", [DC, 128, B_LOC], fp8, kind="ExternalInput").ap()
    b1c = nc.dram_tensor("b1c", [DC, 128, 1], f32, kind="ExternalInput").ap()
    w2c = nc.dram_tensor("w2c", [DC, 128, 1], f16, kind="ExternalInput").ap()
    ident = nc.dram_tensor("ident", [128, 128], f16, kind="ExternalInput").ap()
    out = nc.dram_tensor("out", [B_LOC, D], f32, kind="ExternalOutput").ap()

    with tile.TileContext(nc) as tc, ExitStack() as ctx:
        singles = ctx.enter_context(tc.tile_pool(name="singles", bufs=1))
        encT_pool = ctx.enter_context(tc.tile_pool(name="encTp", bufs=8))
        enc8_pool = ctx.enter_context(tc.tile_pool(name="enc8p", bufs=16))
        ph_pool = ctx.enter_context(tc.tile_pool(name="php", bufs=6, space="PSUM"))
        scsh_ps = ctx.enter_context(tc.tile_pool(name="scshps", bufs=2, space="PSUM"))
        # Emission order drives DMA priority and tile-scheduler priority --
        # front-load exactly the first proj working set (w1e slabs + encT
        # b0) so the PE can start as early as possible; encN is not needed
        # until the context matmuls ~15us later, so those DMAs are emitted
        # after each row's proj work.

        # HAM warm-up: the PE clock-gate sits at 1.2 GHz until it sees
        # ~3.4us of sustained matmul activity.  The PE is idle until the
        # first encT DMA lands (~9us) anyway, so burn that wait on dummy
        # matmuls over a memset tile -- the first real matmuls then issue
        # at the full 2.4 GHz.
        warm_sb = singles.tile([128, 128], f16, name="warm_sb")
        nc.gpsimd.memset(warm_sb, 0.0)
        warm_ps = scsh_ps.tile([128, 128], f32, tag="scsh", name="warm_ps")
        for _ in range(38):
            nc.tensor.matmul(warm_ps[:32, :], lhsT=warm_sb[:, :32],
                             rhs=warm_sb, start=True, stop=True)

        # Spread the first batch of dma_starts across idle engine queues:
        # descriptor-gen is ~0.6us serialized per issuing engine, so putting
        # them all on sync would delay the critical encT[0] transfer.
        w1e_slabs = []
        for hc in range(DC):
            if hc < F16C:
                t = singles.tile([128, DC, 128], f16, name=f"w1e16_{hc}")
                nc.scalar.dma_start(out=t, in_=w1eT16[hc])
            else:
                t = singles.tile([128, DC, 128], fp8, name=f"w1e8_{hc}")
                nc.scalar.dma_start(out=t, in_=w1eT8[hc - F16C])
            w1e_slabs.append(t)
        encT_b0s0 = encT_pool.tile([128, DC, SBLK], f16, tag="encT")
        nc.sync.dma_start(out=encT_b0s0, in_=encT[0, 0])
        dec16_sb = singles.tile([128, DC, B_LOC], f16)
        nc.sync.dma_start(out=dec16_sb, in_=decT16.rearrange("dc p b -> p dc b"))
        dec8_sb = singles.tile([128, DC, B_LOC], fp8)
        nc.sync.dma_start(out=dec8_sb, in_=decT8.rearrange("dc p b -> p dc b"))
        b1_sb = singles.tile([128, DC, 1], f32)
        nc.sync.dma_start(out=b1_sb, in_=b1c.rearrange("dc p o -> p dc o"))
        encT_b0s1 = encT_pool.tile([128, DC, SBLK], f16, tag="encT")
        nc.sync.dma_start(out=encT_b0s1, in_=encT[0, 1])
        w1d_slabs = []
        for hc in range(DC):
            if hc < F16C:
                t = singles.tile([128, DC, 128], f16, name=f"w1d16_{hc}")
                nc.scalar.dma_start(out=t, in_=w1dT16[hc])
            else:
                t = singles.tile([128, DC, 128], fp8, name=f"w1d8_{hc}")
                nc.scalar.dma_start(out=t, in_=w1dT8[hc - F16C])
            w1d_slabs.append(t)
        w2_sb = singles.tile([128, DC, 1], f16)
        nc.scalar.dma_start(out=w2_sb, in_=w2c.rearrange("dc p o -> p dc o"))
        ident_sb = singles.tile([128, 128], f16)
        nc.scalar.dma_start(out=ident_sb, in_=ident)
        pdb1 = singles.tile([128, DC, B_LOC], f32)

        def emit_proj(ph, hc, rhs16, rhs8):
            """Accumulate one [128, N] proj PSUM tile for h-chunk hc."""
            if hc < F16C:
                for dc in range(DC):
                    nc.tensor.matmul(ph, lhsT=w1e_slabs[hc][:, dc, :],
                                     rhs=rhs16[:, dc, :],
                                     start=(dc == 0), stop=(dc == DC - 1))
            else:
                for jp in range(DC // 2):
                    nc.tensor.matmul(ph, lhsT=w1e_slabs[hc][:, 2 * jp:2 * jp + 2, :],
                                     rhs=rhs8[jp],
                                     start=(jp == 0), stop=(jp == DC // 2 - 1),
                                     perf_mode=DR)

        def emit_pd(hc):
            pd_ps = scsh_ps.tile([128, B_LOC], f32, tag="scsh", name=f"pd{hc}")
            if hc < F16C:
                for dc in range(DC):
                    nc.tensor.matmul(pd_ps, lhsT=w1d_slabs[hc][:, dc, :],
                                     rhs=dec16_sb[:, dc, :],
                                     start=(dc == 0), stop=(dc == DC - 1))
            else:
                for jp in range(DC // 2):
                    nc.tensor.matmul(pd_ps, lhsT=w1d_slabs[hc][:, 2 * jp:2 * jp + 2, :],
                                     rhs=dec8_sb[:, 2 * jp:2 * jp + 2, :],
                                     start=(jp == 0), stop=(jp == DC // 2 - 1),
                                     perf_mode=DR)
            nc.scalar.activation(out=pdb1[:, hc, :], in_=pd_ps,
                                 func=AF.Identity, bias=b1_sb[:, hc, :],
                                 scale=1.0)

        encN_pool = ctx.enter_context(tc.tile_pool(name="encNp", bufs=GB + 2))
        hT_pool = ctx.enter_context(tc.tile_pool(name="hTp", bufs=GB * NSB + 2))
        sg_pool = ctx.enter_context(tc.tile_pool(name="sgp", bufs=2))
        small = ctx.enter_context(tc.tile_pool(name="small", bufs=2))
        at_pool = ctx.enter_context(tc.tile_pool(name="atp", bufs=2))
        ctxg_pool = ctx.enter_context(tc.tile_pool(name="ctxgp", bufs=2))

        def emit_row_loads(b):
            """DMA encT[b] and emit the fp16->fp8 DVE casts for it.

            Casts are split per dc-pair so each DoubleRow accumulation pair
            only waits for its own half."""
            if b == 0:
                encT_sbs = [encT_b0s0, encT_b0s1]
            else:
                encT_sbs = []
                for sb in range(NSB):
                    encT_t = encT_pool.tile([128, DC, SBLK], f16, tag="encT",
                                            name="encT_t")
                    nc.sync.dma_start(out=encT_t, in_=encT[b, sb])
                    encT_sbs.append(encT_t)
            enc8_sbs = []
            for sb in range(NSB):
                pairs = []
                for jp in range(DC // 2):
                    e8 = enc8_pool.tile([128, 2, SBLK], fp8, tag="enc8",
                                        name="e8")
                    nc.vector.tensor_copy(
                        out=e8, in_=encT_sbs[sb][:, 2 * jp:2 * jp + 2, :])
                    pairs.append(e8)
                enc8_sbs.append(pairs)
            return encT_sbs, enc8_sbs

        loads = {0: emit_row_loads(0)}
        for g in range(NG):
            # group rows live at partitions {0, 32, 64, 96}: engine writes to
            # a single partition are only legal at 32-aligned bases.
            sc_shs = []
            encN_bs = []
            hT_units = {}
            for bi in range(GB):
                b = g * GB + bi
                if b not in loads:
                    loads[b] = emit_row_loads(b)
                # keep 2 rows of DMA/cast prefetch in flight ahead of the PE
                for bn in (b + 1, b + 2):
                    if bn < B_LOC and bn not in loads:
                        loads[bn] = emit_row_loads(bn)
                encT_sbs, enc8_sbs = loads.pop(b)
                # Batch same-perf-mode matmuls: all fp16 chunks of this row
                # first, then all DoubleRow chunks -- a DR<->normal mode
                # switch costs PE reconfig time, so keep it to 2 per row.
                for sb in range(NSB):
                    hT_units[(bi, sb)] = hT_pool.tile([128, DC, SBLK], f16,
                                                      tag="hT", name="hT")
                # palindrome hc order: odd rows run hc 3..0 so the row
                # boundary keeps the same perf mode (DR meets DR, fp16 meets
                # fp16) -- halves the DR<->normal reconfig count.
                hc_order = range(DC) if b % 2 == 0 else range(DC - 1, -1, -1)
                for hc in hc_order:
                    for sb in range(NSB):
                        ph = ph_pool.tile([128, SBLK], f32, tag="ph")
                        emit_proj(ph, hc, encT_sbs[sb], enc8_sbs[sb])
                        if b == 0 and sb == 0:
                            # must be emitted before the first tanh that
                            # reads pdb1[:, hc] (dependency is emission-order)
                            emit_pd(hc)
                        nc.scalar.activation(out=hT_units[(bi, sb)][:, hc, :],
                                             in_=ph, func=AF.Tanh,
                                             bias=pdb1[:, hc, b:b + 1],
                                             scale=1.0)
                encN_b = encN_pool.tile([128, NSC, D], f16, tag="encN")
                nc.sync.dma_start(out=encN_b, in_=encN[b])
                encN_bs.append(encN_b)

            # Batched scores: one col-tiled PSUM tile per s-block, batch row
            # bi lands at partition 32*bi. Issue round-robin across the four
            # column groups (bi inner) so the 4 accumulation chains stream
            # concurrently on the PE's 32-wide column strips.
            for sb in range(NSB):
                sc_shs.append(scsh_ps.tile([128, SBLK], f32, tag="scsh",
                                           name=f"scsh{g}_{sb}"))
            for hc in range(DC):
                for sb in range(NSB):
                    for bi in range(GB):
                        nc.tensor.matmul(sc_shs[sb][32 * bi:32 * bi + 1, :],
                                         lhsT=w2_sb[:, hc, :],
                                         rhs=hT_units[(bi, sb)][:, hc, :],
                                         start=(hc == 0), stop=(hc == DC - 1),
                                         tile_position=(0, 32 * bi))

            # Scores are O(+-3) for this problem class (weights scaled by
            # 1/sqrt(D)), so exp() cannot overflow -- skip the max
            # subtraction and take exp straight out of the scores PSUM, which
            # removes the whole reduce chain from the critical path.
            den_h = small.tile([128, NSB], f32, tag="den_h")
            alpha_n = sg_pool.tile([128, S], f16, tag="alpha_n")
            alphaT = at_pool.tile([128, NSC, GB], f16, tag="alphaT")
            for e in range(NSB):
                nc.scalar.activation(
                    out=alpha_n[:, e * SBLK:(e + 1) * SBLK],
                    in_=sc_shs[e], func=AF.Exp,
                    bias=0.0, scale=1.0,
                    accum_out=den_h[:, e:e + 1])
            for sc in range(NSC):
                tr_ps = scsh_ps.tile([128, 128], f16, tag="scsh",
                                     name=f"tr{g}_{sc}")
                nc.tensor.transpose(tr_ps,
                                    alpha_n[:, sc * 128:(sc + 1) * 128],
                                    ident_sb)
                # group rows sat at partitions 32*bi -> columns 32*bi after
                # the transpose; gather into a dense tile.
                nc.vector.tensor_copy(
                    out=alphaT[:, sc, :],
                    in_=tr_ps.rearrange("p (g r) -> p g r", g=GB)[:, :, 0])
            den = small.tile([128, 1], f32, tag="den")
            nc.vector.tensor_reduce(out=den, in_=den_h, axis=AX.X,
                                    op=OP.add)
            rden = small.tile([128, 1], f32, tag="rden")
            nc.vector.reciprocal(out=rden, in_=den)

            ctx_g = ctxg_pool.tile([128, D], f32, tag="ctxg")
            ctx_sh = scsh_ps.tile([128, D], f32, tag="scsh",
                                   name=f"ctxsh{g}")
            for sc in range(NSC):
                for bi in range(GB):
                    nc.tensor.matmul(ctx_sh[32 * bi:32 * bi + 1, :],
                                     lhsT=alphaT[:, sc, bi:bi + 1],
                                     rhs=encN_bs[bi][:, sc, :],
                                     start=(sc == 0), stop=(sc == NSC - 1),
                                     tile_position=(0, 32 * bi))
            nc.vector.tensor_scalar(out=ctx_g, in0=ctx_sh, scalar1=rden,
                                    scalar2=None, op0=OP.mult)
            pitch = ctx_g.ap[0][0]
            ctx_rows = bass.AP(tensor=ctx_g.tensor, offset=ctx_g.offset,
                               ap=[[32 * pitch, GB]] +
                                  [list(dd) for dd in ctx_g.ap[1:]])
            nc.sync.dma_start(out=out[g * GB:(g + 1) * GB, :], in_=ctx_rows)

    nc.compile()
    return nc


def _get_nc():
    if "nc" not in _CACHE:
        _CACHE["nc"] = _build()
    return _CACHE["nc"]


def _prep_in_maps(inputs):
    dec = np.asarray(inputs["decoder_hidden"], dtype=np.float32)
    enc = np.asarray(inputs["encoder_outputs"], dtype=np.float32)
    W1 = np.asarray(inputs["W1"], dtype=np.float32)
    b1 = np.asarray(inputs["b1"], dtype=np.float32)
    W2 = np.asarray(inputs["W2"], dtype=np.float32)

    # Permute h-channels so the largest-|w2| rows form chunk 0 (computed in
    # fp16); proj error reaches the output only via sum_h w2_h tanh(.), so
    # protecting the high-|w2| rows cuts the fp8 error ~2x.
    perm = np.argsort(-np.abs(W2[0]))
    W1e_p = W1[:, D:][perm]          # [h, d]
    W1d_p = W1[:, :D][perm]
    b1_p = b1[perm]
    w2_p = W2[0][perm]

    def _slab(wT, dtype):
        # wT [d, h] -> [hc, p, dc, h'] with d = dc*128+p, h = hc*128+h'
        return np.ascontiguousarray(
            wT.reshape(DC, 128, DC, 128).transpose(2, 1, 0, 3)).astype(dtype)

    w1e_slab = _slab(W1e_p.T, np.float32)
    w1d_slab = _slab(W1d_p.T, np.float32)
    w1eT16 = w1e_slab[:F16C].astype(_NPF16)
    w1eT8 = w1e_slab[F16C:].astype(_NPF8)
    w1dT16 = w1d_slab[:F16C].astype(_NPF16)
    w1dT8 = w1d_slab[F16C:].astype(_NPF8)
    b1c = np.ascontiguousarray(b1_p).reshape(DC, 128, 1).astype(np.float32)
    w2c = np.ascontiguousarray(w2_p).reshape(DC, 128, 1).astype(_NPF16)
    ident = np.eye(128, dtype=_NPF16)

    in_maps = []
    for c in range(N_CORES):
        sl = slice(c * B_LOC, (c + 1) * B_LOC)
        enc_c = enc[sl]                                  # [16, 1024, 512]
        # encT[b, sb, p, dc, s] = enc[b, sb*SBLK+s, dc*128+p] -- contiguous
        # 4KB per partition line per DMA (fp16).
        encT_c = np.ascontiguousarray(
            enc_c.reshape(B_LOC, NSB, SBLK, DC, 128).transpose(0, 1, 4, 3, 2)
        ).astype(_NPF16)
        # encN[b, p, sc, d] = enc[b, sc*128+p, d] -- contiguous 8KB lines.
        encN_c = np.ascontiguousarray(
            enc_c.reshape(B_LOC, NSC, 128, D).transpose(0, 2, 1, 3)
        ).astype(_NPF16)
        decT16_c = np.ascontiguousarray(dec[sl].T).reshape(DC, 128, B_LOC) \
            .astype(_NPF16)
        decT8_c = decT16_c.astype(_NPF8)
        in_maps.append({
            "encT": encT_c, "encN": encN_c,
            "w1eT16": w1eT16, "w1eT8": w1eT8,
            "w1dT16": w1dT16, "w1dT8": w1dT8,
            "decT16": decT16_c, "decT8": decT8_c,
            "b1c": b1c, "w2c": w2c, "ident": ident,
        })
    return in_maps


def _run(inputs, trace=False, **kw):
    from concourse.bass_utils import run_bass_kernel_spmd
    nc = _get_nc()
    in_maps = _prep_in_maps(inputs)
    res = run_bass_kernel_spmd(nc, in_maps, core_ids=list(range(N_CORES)),
                               trace=trace, **kw)
    outs = [res.results[i]["out"] for i in range(N_CORES)]
    full = np.concatenate(outs, axis=0).astype(np.float32)
    return full, res


def kernel(**inputs) -> np.ndarray:
    full, _ = _run(inputs, trace=False)
    return full


# revision 28
# speedup vs baseline: 1.0380x; 1.0380x over previous
"""Trainium2 Bass kernel for Bahdanau-style attention (nn_Attention).

Reference computation (B=128, S=1024, D=512):
    proj = tanh(concat(dec, enc) @ W1.T + b1)        # [B, S, D]
    scores = proj @ W2.T (+ b2, cancels in softmax)  # [B, S]
    alpha = softmax(scores, axis=1)
    context = einsum('bs,bsd->bd', alpha, enc)       # [B, D]

Strategy: pure data-parallel over batch (16 rows per NeuronCore, 8 cores).
Per-core dataflow:
  - h-channels are permuted by |w2| (host side): the top-128 rows land in
    chunk hc0 and are computed in fp16; chunks hc1-3 run in fp8-e4m3 with
    DoubleRow perf mode (contracts 256/matmul).  Since proj error reaches
    the output only through scores = sum_h w2_h tanh(proj_h), and the top
    128 |w2| rows carry ~71% of sum(w2^2), this cuts the fp8-induced
    error ~2x (verified rel err ~1.45e-2 vs the 2e-2 budget on the
    harness input set; all-fp8 measured 2.6e-2).
  - encT is DMAed once in fp16; the fp8 copy for the DoubleRow chunks is
    produced on-chip by an idle-DVE tensor_copy (saves 8.4MB of HBM
    traffic).
  - (proj_dec[b] + b1) becomes a per-partition bias fused into the
    ScalarE tanh that evacuates PSUM.
  - scores = W2 . hiddenT via M=1 PE matmuls issued round-robin across
    the four 32-wide PE column groups (tile_position) so the 4 batch
    rows of a softmax group stream concurrently.
  - softmax batched over groups of 4 batch rows (partitions 0/32/64/96):
    Exp reads the scores PSUM directly on ScalarE with a fused accum_out
    denominator; max-subtraction skipped (scores are O(+-3) here).
  - unnormalized alpha (fp16) transposed via PE transpose; context =
    alphaT^T @ enc_natural via col-tiled M=1 matmuls, also round-robin,
    scaled by 1/den at evacuation.
Host side: shard batch, permute h by |w2|, pre-transpose enc to encT
(fp16) + encN (fp16).
"""

import numpy as np
import ml_dtypes

B, S, D = 128, 1024, 512
N_CORES = 8
B_LOC = B // N_CORES          # 16
GB = 4                        # batch rows per softmax group
NG = B_LOC // GB              # 4 groups
DC = D // 128                 # 4 chunks of 128 along d (and h)
F16C = 1                      # h-chunks computed in fp16 (rest fp8 DR)
SBLK = 512                    # s block for proj/score tiles
NSB = S // SBLK               # 2
NSC = S // 128                # 8 s-chunks of 128

_NPF16 = np.float16
_NPF8 = ml_dtypes.float8_e4m3     # TRN float8e4 (max 240, matches ml_dtypes)
_CACHE: dict = {}


def _build():
    from contextlib import ExitStack
    import concourse.bass as bass  # noqa: F401
    import concourse.tile as tile
    from concourse import bacc, mybir

    f32, f16 = mybir.dt.float32, mybir.dt.float16
    fp8 = mybir.dt.float8e4
    AX = mybir.AxisListType
    OP = mybir.AluOpType
    AF = mybir.ActivationFunctionType
    DR = mybir.MatmulPerfMode.DoubleRow

    nc = bacc.Bacc("TRN2", target_bir_lowering=False, debug=False,
                   num_devices=N_CORES)

    encT = nc.dram_tensor("encT", [B_LOC, NSB, 128, DC, SBLK], f16, kind="ExternalInput").ap()
    encN = nc.dram_tensor("encN", [B_LOC, 128, NSC, D], f16, kind="ExternalInput").ap()
    w1eT16 = nc.dram_tensor("w1eT16", [F16C, 128, DC, 128], f16, kind="ExternalInput").ap()
    w1eT8 = nc.dram_tensor("w1eT8", [DC - F16C, 128, DC, 128], fp8, kind="ExternalInput").ap()
    w1dT16 = nc.dram_tensor("w1dT16", [F16C, 128, DC, 128], f16, kind="ExternalInput").ap()
    w1dT8 = nc.dram_tensor("w1dT8", [DC - F16C, 128, DC, 128], fp8, kind="ExternalInput").ap()
    decT16 = nc.dram_tensor("decT16", [DC, 128, B_LOC], f16, kind="ExternalInput").ap()
    decT8 = nc.dram_tensor("decT8", [DC, 128, B_LOC], fp8, kind="ExternalInput").ap()
    b1c = nc.dram_tensor("b1c", [DC, 128, 1], f32, kind="ExternalInput").ap()
    w2c = nc.dram_tensor("w2c", [DC, 128, 1], f16, kind="ExternalInput").ap()
    ident = nc.dram_tensor("ident", [128, 128], f16, kind="ExternalInput").ap()
    out = nc.dram_tensor("out", [B_LOC, D], f32, kind="ExternalOutput").ap()

    with tile.TileContext(nc) as tc, ExitStack() as ctx:
        singles = ctx.enter_context(tc.tile_pool(name="singles", bufs=1))
        encT_pool = ctx.enter_context(tc.tile_pool(name="encTp", bufs=10))
        enc8_pool = ctx.enter_context(tc.tile_pool(name="enc8p", bufs=16))
        ph_pool = ctx.enter_context(tc.tile_pool(name="php", bufs=6, space="PSUM"))
        scsh_ps = ctx.enter_context(tc.tile_pool(name="scshps", bufs=2, space="PSUM"))
        # Emission order drives DMA priority and tile-scheduler priority --
        # front-load exactly the first proj working set (w1e slabs + encT
        # b0) so the PE can start as early as possible; encN is not needed
        # until the context matmuls ~15us later, so those DMAs are emitted
        # after each row's proj work.

        # HAM warm-up: the PE clock-gate sits at 1.2 GHz until it sees
        # ~3.4us of sustained matmul activity.  The PE is idle until the
        # first encT DMA lands (~9us) anyway, so burn that wait on dummy
        # matmuls over a memset tile -- the first real matmuls then issue
        # at the full 2.4 GHz.
        warm_sb = singles.tile([128, 128], f16, name="warm_sb")
        nc.gpsimd.memset(warm_sb, 0.0)
        warm_ps = scsh_ps.tile([128, 128], f32, tag="scsh", name="warm_ps")
        for _ in range(38):
            nc.tensor.matmul(warm_ps[:32, :], lhsT=warm_sb[:, :32],
                             rhs=warm_sb, start=True, stop=True)

        # Spread the first batch of dma_starts across idle engine queues:
        # descriptor-gen is ~0.6us serialized per issuing engine, so putting
        # them all on sync would delay the critical encT[0] transfer.
        w1e_slabs = []
        for hc in range(DC):
            if hc < F16C:
                t = singles.tile([128, DC, 128], f16, name=f"w1e16_{hc}")
                nc.scalar.dma_start(out=t, in_=w1eT16[hc])
            else:
                t = singles.tile([128, DC, 128], fp8, name=f"w1e8_{hc}")
                nc.scalar.dma_start(out=t, in_=w1eT8[hc - F16C])
            w1e_slabs.append(t)
        encT_b0s0 = encT_pool.tile([128, DC, SBLK], f16, tag="encT")
        nc.sync.dma_start(out=encT_b0s0, in_=encT[0, 0])
        dec16_sb = singles.tile([128, DC, B_LOC], f16)
        nc.sync.dma_start(out=dec16_sb, in_=decT16.rearrange("dc p b -> p dc b"))
        dec8_sb = singles.tile([128, DC, B_LOC], fp8)
        nc.sync.dma_start(out=dec8_sb, in_=decT8.rearrange("dc p b -> p dc b"))
        b1_sb = singles.tile([128, DC, 1], f32)
        nc.sync.dma_start(out=b1_sb, in_=b1c.rearrange("dc p o -> p dc o"))
        encT_b0s1 = encT_pool.tile([128, DC, SBLK], f16, tag="encT")
        nc.sync.dma_start(out=encT_b0s1, in_=encT[0, 1])
        w1d_slabs = []
        for hc in range(DC):
            if hc < F16C:
                t = singles.tile([128, DC, 128], f16, name=f"w1d16_{hc}")
                nc.scalar.dma_start(out=t, in_=w1dT16[hc])
            else:
                t = singles.tile([128, DC, 128], fp8, name=f"w1d8_{hc}")
                nc.scalar.dma_start(out=t, in_=w1dT8[hc - F16C])
            w1d_slabs.append(t)
        w2_sb = singles.tile([128, DC, 1], f16)
        nc.scalar.dma_start(out=w2_sb, in_=w2c.rearrange("dc p o -> p dc o"))
        ident_sb = singles.tile([128, 128], f16)
        nc.scalar.dma_start(out=ident_sb, in_=ident)
        pdb1 = singles.tile([128, DC, B_LOC], f32)

        def emit_proj(ph, hc, rhs16, rhs8):
            """Accumulate one [128, N] proj PSUM tile for h-chunk hc."""
            if hc < F16C:
                for dc in range(DC):
                    nc.tensor.matmul(ph, lhsT=w1e_slabs[hc][:, dc, :],
                                     rhs=rhs16[:, dc, :],
                                     start=(dc == 0), stop=(dc == DC - 1))
            else:
                for jp in range(DC // 2):
                    nc.tensor.matmul(ph, lhsT=w1e_slabs[hc][:, 2 * jp:2 * jp + 2, :],
                                     rhs=rhs8[jp],
                                     start=(jp == 0), stop=(jp == DC // 2 - 1),
                                     perf_mode=DR)

        def emit_pd(hc):
            pd_ps = scsh_ps.tile([128, B_LOC], f32, tag="scsh", name=f"pd{hc}")
            if hc < F16C:
                for dc in range(DC):
                    nc.tensor.matmul(pd_ps, lhsT=w1d_slabs[hc][:, dc, :],
                                     rhs=dec16_sb[:, dc, :],
                                     start=(dc == 0), stop=(dc == DC - 1))
            else:
                for jp in range(DC // 2):
                    nc.tensor.matmul(pd_ps, lhsT=w1d_slabs[hc][:, 2 * jp:2 * jp + 2, :],
                                     rhs=dec8_sb[:, 2 * jp:2 * jp + 2, :],
                                     start=(jp == 0), stop=(jp == DC // 2 - 1),
                                     perf_mode=DR)
            nc.scalar.activation(out=pdb1[:, hc, :], in_=pd_ps,
                                 func=AF.Identity, bias=b1_sb[:, hc, :],
                                 scale=1.0)

        encN_pool = ctx.enter_context(tc.tile_pool(name="encNp", bufs=GB + 2))
        hT_pool = ctx.enter_context(tc.tile_pool(name="hTp", bufs=GB * NSB + 2))
        sg_pool = ctx.enter_context(tc.tile_pool(name="sgp", bufs=2))
        small = ctx.enter_context(tc.tile_pool(name="small", bufs=2))
        at_pool = ctx.enter_context(tc.tile_pool(name="atp", bufs=2))
        ctxg_pool = ctx.enter_context(tc.tile_pool(name="ctxgp", bufs=2))

        def emit_row_loads(b):
            """DMA encT[b] and emit the fp16->fp8 DVE casts for it.

            Casts are split per dc-pair so each DoubleRow accumulation pair
            only waits for its own half."""
            if b == 0:
                encT_sbs = [encT_b0s0, encT_b0s1]
            else:
                encT_sbs = []
                for sb in range(NSB):
                    encT_t = encT_pool.tile([128, DC, SBLK], f16, tag="encT",
                                            name="encT_t")
                    nc.sync.dma_start(out=encT_t, in_=encT[b, sb])
                    encT_sbs.append(encT_t)
            enc8_sbs = []
            for sb in range(NSB):
                pairs = []
                for jp in range(DC // 2):
                    e8 = enc8_pool.tile([128, 2, SBLK], fp8, tag="enc8",
                                        name="e8")
                    nc.vector.tensor_copy(
                        out=e8, in_=encT_sbs[sb][:, 2 * jp:2 * jp + 2, :])
                    pairs.append(e8)
                enc8_sbs.append(pairs)
            return encT_sbs, enc8_sbs

        loads = {0: emit_row_loads(0)}
        for g in range(NG):
            # group rows live at partitions {0, 32, 64, 96}: engine writes to
            # a single partition are only legal at 32-aligned bases.
            sc_shs = []
            encN_bs = []
            hT_units = {}
            for bi in range(GB):
                b = g * GB + bi
                if b not in loads:
                    loads[b] = emit_row_loads(b)
                # keep 3 rows of DMA/cast prefetch in flight ahead of the PE
                for bn in (b + 1, b + 2, b + 3):
                    if bn < B_LOC and bn not in loads:
                        loads[bn] = emit_row_loads(bn)
                encT_sbs, enc8_sbs = loads.pop(b)
                # Batch same-perf-mode matmuls: all fp16 chunks of this row
                # first, then all DoubleRow chunks -- a DR<->normal mode
                # switch costs PE reconfig time, so keep it to 2 per row.
                for sb in range(NSB):
                    hT_units[(bi, sb)] = hT_pool.tile([128, DC, SBLK], f16,
                                                      tag="hT", name="hT")
                for hc in range(DC):
                    for sb in range(NSB):
                        ph = ph_pool.tile([128, SBLK], f32, tag="ph")
                        emit_proj(ph, hc, encT_sbs[sb], enc8_sbs[sb])
                        if b == 0 and sb == 0:
                            # must be emitted before the first tanh that
                            # reads pdb1[:, hc] (dependency is emission-order)
                            emit_pd(hc)
                        nc.scalar.activation(out=hT_units[(bi, sb)][:, hc, :],
                                             in_=ph, func=AF.Tanh,
                                             bias=pdb1[:, hc, b:b + 1],
                                             scale=1.0)
                encN_b = encN_pool.tile([128, NSC, D], f16, tag="encN")
                nc.sync.dma_start(out=encN_b, in_=encN[b])
                encN_bs.append(encN_b)

            # Batched scores: one col-tiled PSUM tile per s-block, batch row
            # bi lands at partition 32*bi. Issue round-robin across the four
            # column groups (bi inner) so the 4 accumulation chains stream
            # concurrently on the PE's 32-wide column strips.
            for sb in range(NSB):
                sc_shs.append(scsh_ps.tile([128, SBLK], f32, tag="scsh",
                                           name=f"scsh{g}_{sb}"))
            for hc in range(DC):
                for sb in range(NSB):
                    for bi in range(GB):
                        nc.tensor.matmul(sc_shs[sb][32 * bi:32 * bi + 1, :],
                                         lhsT=w2_sb[:, hc, :],
                                         rhs=hT_units[(bi, sb)][:, hc, :],
                                         start=(hc == 0), stop=(hc == DC - 1),
                                         tile_position=(0, 32 * bi))

            # Scores are O(+-3) for this problem class (weights scaled by
            # 1/sqrt(D)), so exp() cannot overflow -- skip the max
            # subtraction and take exp straight out of the scores PSUM, which
            # removes the whole reduce chain from the critical path.
            den_h = small.tile([128, NSB], f32, tag="den_h")
            alpha_n = sg_pool.tile([128, S], f16, tag="alpha_n")
            alphaT = at_pool.tile([128, NSC, GB], f16, tag="alphaT")
            for e in range(NSB):
                nc.scalar.activation(
                    out=alpha_n[:, e * SBLK:(e + 1) * SBLK],
                    in_=sc_shs[e], func=AF.Exp,
                    bias=0.0, scale=1.0,
                    accum_out=den_h[:, e:e + 1])
            for sc in range(NSC):
                tr_ps = scsh_ps.tile([128, 128], f16, tag="scsh",
                                     name=f"tr{g}_{sc}")
                nc.tensor.transpose(tr_ps,
                                    alpha_n[:, sc * 128:(sc + 1) * 128],
                                    ident_sb)
                # group rows sat at partitions 32*bi -> columns 32*bi after
                # the transpose; gather into a dense tile.
                nc.vector.tensor_copy(
                    out=alphaT[:, sc, :],
                    in_=tr_ps.rearrange("p (g r) -> p g r", g=GB)[:, :, 0])
            den = small.tile([128, 1], f32, tag="den")
            nc.vector.tensor_reduce(out=den, in_=den_h, axis=AX.X,
                                    op=OP.add)
            rden = small.tile([128, 1], f32, tag="rden")
            nc.vector.reciprocal(out=rden, in_=den)

            ctx_g = ctxg_pool.tile([128, D], f32, tag="ctxg")
            ctx_sh = scsh_ps.tile([128, D], f32, tag="scsh",
                                   name=f"ctxsh{g}")
            for sc in range(NSC):
                for bi in range(GB):
                    nc.tensor.matmul(ctx_sh[32 * bi:32 * bi + 1, :],
                                     lhsT=alphaT[:, sc, bi:bi + 1],
                                     rhs=encN_bs[bi][:, sc, :],
                                     start=(sc == 0), stop=(sc == NSC - 1),
                                     tile_position=(0, 32 * bi))
            nc.vector.tensor_scalar(out=ctx_g, in0=ctx_sh, scalar1=rden,
                                    scalar2=None, op0=OP.mult)
            pitch = ctx_g.ap[0][0]
            ctx_rows = bass.AP(tensor=ctx_g.tensor, offset=ctx_g.offset,
                               ap=[[32 * pitch, GB]] +
                                  [list(dd) for dd in ctx_g.ap[1:]])
            nc.sync.dma_start(out=out[g * GB:(g + 1) * GB, :], in_=ctx_rows)

    nc.compile()
    return nc


def _get_nc():
    if "nc" not in _CACHE:
        _CACHE["nc"] = _build()
    return _CACHE["nc"]


def _prep_in_maps(inputs):
    dec = np.asarray(inputs["decoder_hidden"], dtype=np.float32)
    enc = np.asarray(inputs["encoder_outputs"], dtype=np.float32)
    W1 = np.asarray(inputs["W1"], dtype=np.float32)
    b1 = np.asarray(inputs["b1"], dtype=np.float32)
    W2 = np.asarray(inputs["W2"], dtype=np.float32)

    # Permute h-channels so the largest-|w2| rows form chunk 0 (computed in
    # fp16); proj error reaches the output only via sum_h w2_h tanh(.), so
    # protecting the high-|w2| rows cuts the fp8 error ~2x.
    perm = np.argsort(-np.abs(W2[0]))
    W1e_p = W1[:, D:][perm]          # [h, d]
    W1d_p = W1[:, :D][perm]
    b1_p = b1[perm]
    w2_p = W2[0][perm]

    def _slab(wT, dtype):
        # wT [d, h] -> [hc, p, dc, h'] with d = dc*128+p, h = hc*128+h'
        return np.ascontiguousarray(
            wT.reshape(DC, 128, DC, 128).transpose(2, 1, 0, 3)).astype(dtype)

    w1e_slab = _slab(W1e_p.T, np.float32)
    w1d_slab = _slab(W1d_p.T, np.float32)
    w1eT16 = w1e_slab[:F16C].astype(_NPF16)
    w1eT8 = w1e_slab[F16C:].astype(_NPF8)
    w1dT16 = w1d_slab[:F16C].astype(_NPF16)
    w1dT8 = w1d_slab[F16C:].astype(_NPF8)
    b1c = np.ascontiguousarray(b1_p).reshape(DC, 128, 1).astype(np.float32)
    w2c = np.ascontiguousarray(w2_p).reshape(DC, 128, 1).astype(_NPF16)
    ident = np.eye(128, dtype=_NPF16)

    in_maps = []
    for c in range(N_CORES):
        sl = slice(c * B_LOC, (c + 1) * B_LOC)
        enc_c = enc[sl]                                  # [16, 1024, 512]
        # encT[b, sb, p, dc, s] = enc[b, sb*SBLK+s, dc*128+p] -- contiguous
        # 4KB per partition line per DMA (fp16).
        encT_c = np.ascontiguousarray(
            enc_c.reshape(B_LOC, NSB, SBLK, DC, 128).transpose(0, 1, 4, 3, 2)
        ).astype(_NPF16)
        # encN[b, p, sc, d] = enc[b, sc*128+p, d] -- contiguous 8KB lines.
        encN_c = np.ascontiguousarray(
            enc_c.reshape(B_LOC, NSC, 128, D).transpose(0, 2, 1, 3)
        ).astype(_NPF16)
        decT16_c = np.ascontiguousarray(dec[sl].T).reshape(DC, 128, B_LOC) \
            .astype(_NPF16)
        decT8_c = decT16_c.astype(_NPF8)
        in_maps.append({
            "encT": encT_c, "encN": encN_c,
            "w1eT16": w1eT16, "w1eT8": w1eT8,
            "w1dT16": w1dT16, "w1dT8": w1dT8,
            "decT16": decT16_c, "decT8": decT8_c,
            "b1c": b1c, "w2c": w2c, "ident": ident,
        })
    return in_maps


def _run(inputs, trace=False, **kw):
    from concourse.bass_utils import run_bass_kernel_spmd
    nc = _get_nc()
    in_maps = _prep_in_maps(inputs)
    res = run_bass_kernel_spmd(nc, in_maps, core_ids=list(range(N_CORES)),
                               trace=trace, **kw)
    outs = [res.results[i]["out"] for i in range(N_CORES)]
    full = np.concatenate(outs, axis=0).astype(np.float32)
    return full, res


def kernel(**inputs) -> np.ndarray:
    full, _ = _run(inputs, trace=False)
    return full
